# revision 9
# baseline (speedup 1.0000x reference)
"""Trainium2 Bass kernel for nn_DynamLinear: per-codebook linear -> chunked
outer product -> mean over codebooks -> RMS norm.

Math notes:
  ref: y = einsum('td,hdo->tho', x, W); split o=64 into a=y[..., :32], b=y[..., 32:]
       op[t,h,i,j] = a[t,h,i]*b[t,h,j];  out = mean_h(op)*sqrt(16); rms_norm(out)
  Since rms_norm is scale invariant, out = S / sqrt(mean(S^2) + 16e-12) where
       S[t,i,j] = sum_h a[t,h,i]*b[t,h,j]  (the per-token 16x32^T @ 16x32 matmul)

Per-core plan (tokens sharded 1024/core), tuned for the tile-sim cost model
(DMA queue occupancy = free-bytes-per-partition * 0.3855ns; matmul = out free
size * pe_cycle * cycles_per_row):
  stage1: y^T = Wp^T @ x^T on TensorE (bf16), CT-pair psum tiles, ACT copy
          to y_sb.
  shuffle: per (quarter r, m): DMA y_sb[16m:16m+16, :, tq] ->
          z4[32r:32r+16, sel, ctp, m, q]  (16 DMAs sync / 16 gpsimd).
  stage2: per token one matmul lhsT=A_t[16h x 32i], rhs=B_t[16h x 32j] on a
          32x32 PE tile (row r, col q%4) -> PSUM S_t[32i x 32j].
  rms:    ACT square (bf16 out), DVE reduce (2x on bf16), indicator-matmul
          over i, sqrt+reciprocal, DVE mul -> bf16 zout, 8 coarse bf16
          stores on gpsimd (host widens to f32).
"""

import sys
import functools
from contextlib import ExitStack

import numpy as np
import ml_dtypes

if "/opt/trn_rl_repo" not in sys.path:
    sys.path.insert(0, "/opt/trn_rl_repo")

import concourse.bass as bass
import concourse.bacc as bacc
import concourse.tile as tile
from concourse import mybir
from concourse.bass_utils import run_bass_kernel_spmd

N_CORES = 8
T_CORE = 1024          # tokens per core
D = 1024               # feat dim
H = 16                 # codebooks
EPS = 16e-12           # 16 * 1e-12 (scale-folded reference eps)

F32 = mybir.dt.float32
F32R = mybir.dt.float32r
BF16 = mybir.dt.bfloat16
F8 = mybir.dt.float8e4
DR = mybir.MatmulPerfMode.DoubleRow


def _kernel_body(tc, ctx, xt, wp, ind, out):
    nc = tc.nc

    singles = ctx.enter_context(tc.tile_pool(name="singles", bufs=1))
    psum1 = ctx.enter_context(tc.tile_pool(name="psum1", bufs=2, space="PSUM"))
    psum2 = ctx.enter_context(tc.tile_pool(name="psum2", bufs=5, space="PSUM"))
    psum3 = ctx.enter_context(tc.tile_pool(name="psum3", bufs=1, space="PSUM"))
    scratch = ctx.enter_context(tc.tile_pool(name="scratch", bufs=3))
    smalls = ctx.enter_context(tc.tile_pool(name="smalls", bufs=6))

    # ---- resident tiles --------------------------------------------------
    # fp8 hi/lo split of W (x16) and x, k-tiles paired for DoubleRow
    wp_hi = singles.tile([128, 8, 4, 2, 128], F8)  # [dp, CT, dd, k2, c7]
    wp_lo = singles.tile([128, 8, 4, 2, 128], F8)
    xt_hi = singles.tile([128, 4, 2, 1024], F8)    # [dp, dd, k2, t]
    xt_lo = singles.tile([128, 4, 2, 1024], F8)
    y_sb = singles.tile([128, 8, 1024], BF16)     # [p, CT, t]: y^T
    z4 = singles.tile([128, 2, 4, 8, 256], BF16)  # [32r+h, sel, ctp, m, q]
    zout = singles.tile([128, 16, 16, 32], BF16)  # [32c+i, chunk, t32, j]
    ind_sb = singles.tile([128, 128], F32R)       # block-diag ones (4x 32x32)
    eps_sb = singles.tile([128, 1], F32)
    out8 = out.rearrange("p (s f) -> p s f", s=8)

    nc.vector.memset(eps_sb[:], EPS)
    nc.gpsimd.dma_start(out=ind_sb[:], in_=ind[:])

    # ---- input DMAs: wp on scalar queue, xt on sync queue ----------------
    nc.scalar.dma_start(out=wp_hi[:, 0:2], in_=wp[0][:, 0:2])
    nc.scalar.dma_start(out=wp_hi[:, 2:4], in_=wp[0][:, 2:4])
    nc.scalar.dma_start(out=wp_lo[:, 0:4], in_=wp[1][:, 0:4])
    nc.scalar.dma_start(out=wp_hi[:, 4:8], in_=wp[0][:, 4:8])
    nc.scalar.dma_start(out=wp_lo[:, 4:8], in_=wp[1][:, 4:8])
    for r in range(4):
        nc.sync.dma_start(out=xt_hi[:, :, :, 256 * r:256 * r + 256],
                          in_=xt[0][:, :, :, 256 * r:256 * r + 256])
        nc.sync.dma_start(out=xt_lo[:, :, :, 256 * r:256 * r + 256],
                          in_=xt[1][:, :, :, 256 * r:256 * r + 256])

    # ---- stage 1 for one token quarter (fp8 hi/lo DoubleRow) -------------
    def _stage1(r):
        t0 = 256 * r
        for cp in range(4):
            ps = psum1.tile([128, 2, 256], F32)
            for c2 in range(2):
                ct = 2 * cp + c2
                terms = [(wp_hi, xt_hi, dd) for dd in range(4)]
                terms += [(wp_lo, xt_hi, dd) for dd in range(4)]
                terms += [(wp_hi, xt_lo, dd) for dd in range(4)]
                for n, (wl, xl, dd) in enumerate(terms):
                    nc.tensor.matmul(
                        ps[:, c2],
                        lhsT=wl[:, ct, dd],
                        rhs=xl[:, dd, :, t0:t0 + 256],
                        start=(n == 0),
                        stop=(n == 11),
                        perf_mode=DR,
                    )
            nc.scalar.activation(
                y_sb[:, 2 * cp:2 * cp + 2, t0:t0 + 256], ps[:],
                mybir.ActivationFunctionType.Copy,
            )

    # ---- shuffle quarter r: y_sb -> z4 (8 DMAs, 2 queues) ----------------
    def _shuffle(r):
        t0 = 256 * r
        for m in range(8):
            eng = nc.sync if (m % 2 == 0) else nc.gpsimd
            eng.dma_start(
                out=z4[32 * r:32 * r + 16, :, :, m, :],
                in_=y_sb[16 * m:16 * m + 16, :, t0:t0 + 256],
            )

    # ---- stage 2 + rms for the 4 chunks (64 tokens each) of quarter r ----
    def _stage2(r):
        for half in range(4):
            ch = 4 * r + half
            t0 = 64 * half
            ps2 = psum2.tile([128, 16, 32], F32)
            for tw in range(64):
                c, t32 = tw % 4, tw // 4
                t256 = t0 + tw
                nc.tensor.matmul(
                    ps2[32 * c:32 * c + 32, t32, :],
                    lhsT=z4[32 * r:32 * r + 16, 0, :, :, t256],
                    rhs=z4[32 * r:32 * r + 16, 1, :, :, t256],
                    start=True, stop=True,
                    tile_position=(32 * r, 32 * c),
                )
            sq = scratch.tile([128, 16, 32], BF16)
            nc.scalar.square(sq[:], ps2[:])
            part = smalls.tile([128, 16], F32R)
            with nc.allow_low_precision(reason="f32r sum of 32 sq for rms"):
                nc.vector.tensor_reduce(part[:], sq[:],
                                        axis=mybir.AxisListType.X,
                                        op=mybir.AluOpType.add)
            ps3 = psum3.tile([128, 16], F32)
            nc.tensor.matmul(ps3[:], lhsT=ind_sb[:], rhs=part[:],
                             start=True, stop=True)
            s_sb = smalls.tile([128, 16], F32)
            nc.scalar.activation(s_sb[:], ps3[:],
                                 mybir.ActivationFunctionType.Sqrt,
                                 bias=eps_sb[:], scale=1.0 / 1024.0)
            rstd = smalls.tile([128, 16], F32)
            nc.vector.reciprocal(rstd[:], s_sb[:])
            nc.vector.tensor_mul(zout[:, ch], ps2[:],
                                 rstd[:].unsqueeze(2).broadcast_to([128, 16, 32]))
            if ch % 2 == 1:
                nc.sync.dma_start(
                    out=out8[:, ch // 2],
                    in_=zout[:, ch - 1:ch + 1].rearrange("p a b c -> p (a b c)"),
                )

    for r in range(4):
        _stage1(r)
        _shuffle(r)
        if r > 0:
            _stage2(r - 1)
    _stage2(3)


@functools.lru_cache(maxsize=1)
def _build_program():
    nc = bacc.Bacc("TRN2", target_bir_lowering=False, debug=False)
    xt = [nc.dram_tensor(f"xt{s}", [128, 4, 2, 1024], F8,
                         kind="ExternalInput").ap() for s in "hl"]
    wp = [nc.dram_tensor(f"wp{s}", [128, 8, 4, 2, 128], F8,
                         kind="ExternalInput").ap() for s in "hl"]
    ind = nc.dram_tensor("ind", [128, 128], F32R, kind="ExternalInput").ap()
    out = nc.dram_tensor("out", [128, 8192], BF16, kind="ExternalOutput").ap()
    with tile.TileContext(nc) as tc:
        with ExitStack() as ctx:
            _kernel_body(tc, ctx, xt, wp, ind, out)
    nc.compile()
    return nc


NPF8 = mybir.dt.np(F8)


def _split8(a):
    hi = a.astype(NPF8)
    lo = (a - hi.astype(np.float32)).astype(NPF8)
    return hi, lo


def _host_prep(x, weight):
    xf = np.ascontiguousarray(x.reshape(-1, D))          # [8192, 1024]
    # Wp column order: col = 512*sel + 128*ctp + 16*m + h ; i = 8*ctp + m
    w = weight.transpose(1, 0, 2).reshape(D, H, 2, 4, 8)  # [d, h, sel, ctp, m]
    wp = w.transpose(0, 2, 3, 4, 1).reshape(D, 1024)      # [d, col]
    # scale W x16 into fp8-friendly range; rms-norm makes the output invariant
    hi, lo = _split8(wp * 16.0)
    # [d, col] -> [p, ct, dd, k2, c] with d = 128*(2*dd+k2) + p
    wp_sb = [np.ascontiguousarray(
        a.reshape(4, 2, 128, 8, 128).transpose(2, 3, 0, 1, 4))
        for a in (hi, lo)]
    ind = np.kron(np.eye(4, dtype=np.float32),
                  np.ones((32, 32), dtype=np.float32))
    xt_shards = []
    for c in range(N_CORES):
        xtc = xf[c * T_CORE:(c + 1) * T_CORE].T           # [d, t]
        xhi, xlo = _split8(xtc)
        xt_shards.append([np.ascontiguousarray(
            a.reshape(4, 2, 128, 1024).transpose(2, 0, 1, 3))
            for a in (xhi, xlo)])
    return xt_shards, wp_sb, ind


def kernel(x, weight, **_unused):
    x = np.asarray(x, dtype=np.float32)
    weight = np.asarray(weight, dtype=np.float32)
    xt_shards, wp_sb, ind = _host_prep(x, weight)
    nc = _build_program()
    in_maps = [{"xth": xt_shards[c][0], "xtl": xt_shards[c][1],
                "wph": wp_sb[0], "wpl": wp_sb[1], "ind": ind}
               for c in range(N_CORES)]
    res = run_bass_kernel_spmd(nc, in_maps, list(range(N_CORES)))
    outs = []
    for c in range(N_CORES):
        d = np.asarray(res.results[c]["out"]).astype(np.float32)
        d = d.reshape(4, 32, 16, 16, 32)
        # [cg, i, ch, t32, j] -> token t = 64*ch + 4*t32 + cg, row = i*32+j
        outs.append(d.transpose(2, 3, 0, 1, 4).reshape(T_CORE, 1024))
    full = np.concatenate(outs, axis=0)                   # [8192, 1024]
    return full.reshape(x.shape[0], x.shape[1], 1024).astype(np.float32)


if __name__ == "__main__":
    rng = np.random.default_rng(0)
    x = rng.standard_normal((4, 2048, D), dtype=np.float32)
    w = (rng.standard_normal((H, D, 64), dtype=np.float32)
         * np.sqrt(2.0 / (D + 64))).astype(np.float32)
    o = kernel(x, w)
    print(o.shape, o.dtype)
